# revision 3
# baseline (speedup 1.0000x reference)
"""KnowledgeGraphReasoner kernel for 8 Trainium2 NeuronCores.

scores[b, n] = -(||hr_b||^2 - 2 hr_b . E_n + ||E_n||^2)
             = 2 hr_b . E_n - ||hr_b||^2 - ||E_n||^2
predictions[b] = argmax_n scores[b, n]

Sharding: entity axis split across 8 cores (12500 entities each).
hr = E[heads] + R[rels] is tiny ([1024, 256]) and computed host-side,
replicated to every core. Each core runs a [1024 x 12500 x 256] GEMM
(bf16 inputs, fp32 PSUM accumulate). Both norm corrections are folded
so no vector-engine work is needed per tile:
  - ent_sq rides the GEMM as one extra K=1 fp16 matmul
    (ones[1,128]^T @ -ent_sq[1,500] accumulated into the same PSUM)
  - hr_sq is a per-partition bias on the scalar-engine PSUM->SBUF copy
Predictions are computed host-side from the gathered scores, with an
exact rescoring pass over near-max candidates so low-precision GEMM
noise cannot flip the argmax.
"""

import numpy as np
import ml_dtypes

import concourse.bacc as bacc
import concourse.mybir as mybir
import concourse.tile as tile
from concourse.bass_utils import run_bass_kernel_spmd

N_ENT = 100000
D = 256
B = 1024
N_CORES = 8
N_SH = N_ENT // N_CORES          # 12500 entities per core
ETILE = 500                      # entity tile (free dim; <=512 fp32 PSUM)
N_ETILES = N_SH // ETILE         # 25
EGROUP = 2                       # e-tiles per output tile / DMA
P = 128
N_BTILES = B // P                # 8

_compiled = {}


def _build_bass():
    nc = bacc.Bacc("TRN2", target_bir_lowering=False, debug=False,
                   num_devices=N_CORES)
    bf16 = mybir.dt.bfloat16
    f16 = mybir.dt.float16
    f32 = mybir.dt.float32

    et = nc.dram_tensor("et", [D, N_SH], bf16, kind="ExternalInput").ap()
    hrt2 = nc.dram_tensor("hrt2", [D, B], bf16, kind="ExternalInput").ap()
    hrsqn = nc.dram_tensor("hrsqn", [P, N_BTILES], f32, kind="ExternalInput").ap()
    entsqn = nc.dram_tensor("entsqn", [1, N_SH], f16, kind="ExternalInput").ap()
    ones = nc.dram_tensor("ones", [1, P], f16, kind="ExternalInput").ap()
    scores = nc.dram_tensor("scores", [B, N_SH], f32, kind="ExternalOutput").ap()

    # d index = k*128 + p  ->  [p, k, n] view for SBUF tiles
    et_v = et.rearrange("(k p) n -> p k n", p=P)
    hrt2_v = hrt2.rearrange("(k p) b -> p k b", p=P)

    Ident = mybir.ActivationFunctionType.Identity

    with tile.TileContext(nc) as tc:
        with (
            tc.tile_pool(name="const", bufs=1) as cpool,
            tc.tile_pool(name="etp", bufs=4) as etp,
            tc.tile_pool(name="outp", bufs=16) as outp,
            tc.tile_pool(name="psp", bufs=8, space="PSUM") as psp,
        ):
            # resident: hr^T (x2 folded in), [128, 2, 1024] bf16
            hrt_sb = cpool.tile([P, 2, B], bf16, tag="hrt")
            nc.sync.dma_start(hrt_sb[:], hrt2_v[:])
            # resident: per-b-tile -||hr||^2, [128, 8] f32
            hrsqn_sb = cpool.tile([P, N_BTILES], f32, tag="hrsq")
            nc.sync.dma_start(hrsqn_sb[:], hrsqn[:])
            # resident: -||E_n||^2 row, [1, 12500] f16
            entsqn_sb = cpool.tile([1, N_SH], f16, tag="entsq")
            nc.sync.dma_start(entsqn_sb[:], entsqn[:])
            # resident: ones row for the K=1 matmul, [1, 128] f16
            ones_sb = cpool.tile([1, P], f16, tag="ones")
            nc.sync.dma_start(ones_sb[:], ones[:])

            n_groups = (N_ETILES + EGROUP - 1) // EGROUP
            for egg in range(n_groups):
                egs = list(range(egg * EGROUP, min((egg + 1) * EGROUP, N_ETILES)))
                W = len(egs) * ETILE
                base = egs[0] * ETILE
                et_t = etp.tile([P, 2, W], bf16, tag="et")
                nc.sync.dma_start(et_t[:], et_v[:, :, base:base + W])
                ots = [outp.tile([P, W], f32, tag="ot", name=f"ot{egg}_{bt}")
                       for bt in range(N_BTILES)]
                for j, eg in enumerate(egs):
                    esl = slice(eg * ETILE, (eg + 1) * ETILE)
                    jsl = slice(j * ETILE, (j + 1) * ETILE)
                    for bt in range(N_BTILES):
                        bsl = slice(bt * P, (bt + 1) * P)
                        ps = psp.tile([P, ETILE], f32, tag="ps")
                        nc.tensor.matmul(ps[:], hrt_sb[:, 0, bsl],
                                         et_t[:, 0, jsl],
                                         start=True, stop=False)
                        nc.tensor.matmul(ps[:], hrt_sb[:, 1, bsl],
                                         et_t[:, 1, jsl],
                                         start=False, stop=False)
                        nc.tensor.matmul(ps[:], ones_sb[:, :],
                                         entsqn_sb[0:1, esl],
                                         start=False, stop=True)
                        nc.scalar.activation(ots[bt][:, jsl], ps[:], Ident,
                                             bias=hrsqn_sb[:, bt:bt + 1],
                                             scale=1.0)
                for bt in range(N_BTILES):
                    bsl = slice(bt * P, (bt + 1) * P)
                    nc.sync.dma_start(scores[bsl, base:base + W], ots[bt][:])
    nc.compile()
    return nc


def _get_bass():
    if "nc" not in _compiled:
        _compiled["nc"] = _build_bass()
    return _compiled["nc"]


def _device_scores(E, hr, hr_sq, ent_sq, trace=False):
    """Run the sharded GEMM on 8 cores; returns (scores [B, N_ENT], results)."""
    bf = ml_dtypes.bfloat16
    hrt2 = np.ascontiguousarray((2.0 * hr).T).astype(bf)         # [256, 1024]
    hrsqn_t = np.ascontiguousarray((-hr_sq).reshape(N_BTILES, P).T)  # [128, 8]
    ones = np.ones((1, P), dtype=np.float16)
    in_maps = []
    for c in range(N_CORES):
        Ec = E[c * N_SH:(c + 1) * N_SH]
        in_maps.append({
            "et": np.ascontiguousarray(Ec.T).astype(bf),          # [256, 12500]
            "hrt2": hrt2,
            "hrsqn": hrsqn_t,
            "entsqn": (-ent_sq[c * N_SH:(c + 1) * N_SH]
                       ).reshape(1, N_SH).astype(np.float16),
            "ones": ones,
        })
    nc = _get_bass()
    res = run_bass_kernel_spmd(nc, in_maps, core_ids=list(range(N_CORES)),
                               trace=trace)
    scores = np.concatenate([r["scores"] for r in res.results], axis=1)
    return scores, res


def _refine_predictions(scores, hr, E, hr_sq, ent_sq):
    """argmax with exact rescoring of near-max candidates.

    Device scores carry bf16/fp16 input-rounding noise (~0.3 abs). Any
    entity within MARGIN of a query's device max is rescored exactly in
    fp64 so the returned argmax matches an exact computation.
    """
    MARGIN = 2.0
    amax = scores.max(axis=1)
    preds = np.empty(B, dtype=np.int64)
    hr64 = hr.astype(np.float64)
    for b in range(B):
        cand = np.flatnonzero(scores[b] >= amax[b] - MARGIN)
        if cand.size == 1:
            preds[b] = cand[0]
            continue
        s = 2.0 * (E[cand].astype(np.float64) @ hr64[b]) \
            - hr_sq[b] - ent_sq[cand]
        preds[b] = cand[int(np.argmax(s))]
    return preds


def kernel(queries, entity_emb, relation_emb, trace=False):
    queries = np.asarray(queries)
    E = np.ascontiguousarray(np.asarray(entity_emb, dtype=np.float32))
    R = np.ascontiguousarray(np.asarray(relation_emb, dtype=np.float32))
    heads = queries[:, 0].astype(np.int64)
    rels = queries[:, 1].astype(np.int64)

    hr = E[heads] + R[rels]                         # [B, D] f32
    hr_sq = np.sum(hr * hr, axis=1)                 # [B] f32
    ent_sq = np.sum(E * E, axis=1)                  # [N] f32

    scores, res = _device_scores(E, hr, hr_sq, ent_sq, trace=trace)
    preds = _refine_predictions(scores, hr, E, hr_sq, ent_sq)
    if trace:
        kernel.last_results = res
    return scores, preds


# revision 7
# speedup vs baseline: 1.7049x; 1.7049x over previous
"""KnowledgeGraphReasoner kernel for 8 Trainium2 NeuronCores.

scores[b, n] = -(||hr_b||^2 - 2 hr_b . E_n + ||E_n||^2)
             = 2 hr_b . E_n - ||hr_b||^2 - ||E_n||^2
predictions[b] = argmax_n scores[b, n]

Sharding: entity axis split across 8 cores (12500 entities each).
hr = E[heads] + R[rels] is tiny ([1024, 256]) and computed host-side,
replicated to every core. Each core runs a [1024 x 12500 x 256] GEMM
(bf16 inputs, fp32 PSUM accumulate). The norm corrections (exact fp32,
computed host-side) are applied by one fused DVE op per tile:
    out = (psum - hr_sq[b]) - ent_sq[n]
with ent_sq partition-broadcast once per entity group on GpSimd.
Predictions are computed host-side from the gathered scores, with an
exact rescoring pass over near-max candidates so low-precision GEMM
noise cannot flip the argmax.
"""

import numpy as np
import ml_dtypes

import concourse.bacc as bacc
import concourse.mybir as mybir
import concourse.tile as tile
from concourse.bass_utils import run_bass_kernel_spmd

N_ENT = 100000
D = 256
B = 1024
N_CORES = 8
N_SH = N_ENT // N_CORES          # 12500 entities per core
ETILE = 500                      # entity tile (free dim; <=512 fp32 PSUM)
N_ETILES = N_SH // ETILE         # 25
EGROUP = 2                       # e-tiles per output tile / DMA
P = 128
N_BTILES = B // P                # 8

_compiled = {}


def _build_bass():
    nc = bacc.Bacc("TRN2", target_bir_lowering=False, debug=False,
                   num_devices=N_CORES)
    bf16 = mybir.dt.bfloat16
    f32 = mybir.dt.float32

    et = nc.dram_tensor("et", [D, N_SH], bf16, kind="ExternalInput").ap()
    hrt2 = nc.dram_tensor("hrt2", [D, B], bf16, kind="ExternalInput").ap()
    hrsq = nc.dram_tensor("hrsq", [P, N_BTILES], f32, kind="ExternalInput").ap()
    entsq = nc.dram_tensor("entsq", [1, N_SH], f32, kind="ExternalInput").ap()
    scores = nc.dram_tensor("scores", [B, N_SH], f32, kind="ExternalOutput").ap()

    # d index = k*128 + p  ->  [p, k, n] view for SBUF tiles
    et_v = et.rearrange("(k p) n -> p k n", p=P)
    hrt2_v = hrt2.rearrange("(k p) b -> p k b", p=P)

    with tile.TileContext(nc) as tc:
        with (
            tc.tile_pool(name="const", bufs=1) as cpool,
            tc.tile_pool(name="etp", bufs=4) as etp,
            tc.tile_pool(name="ebcp", bufs=3) as ebcp,
            tc.tile_pool(name="outp", bufs=16) as outp,
            tc.tile_pool(name="psp", bufs=8, space="PSUM") as psp,
        ):
            # resident: hr^T (x2 folded in), [128, 2, 1024] bf16
            hrt_sb = cpool.tile([P, 2, B], bf16, tag="hrt")
            nc.sync.dma_start(hrt_sb[:], hrt2_v[:])
            # resident: per-b-tile ||hr||^2, [128, 8] f32
            hrsq_sb = cpool.tile([P, N_BTILES], f32, tag="hrsq")
            nc.sync.dma_start(hrsq_sb[:], hrsq[:])
            # resident: ||E_n||^2 row, [1, 12500] f32
            entsq_sb = cpool.tile([1, N_SH], f32, tag="entsq")
            nc.sync.dma_start(entsq_sb[:], entsq[:])

            n_groups = (N_ETILES + EGROUP - 1) // EGROUP
            for egg in range(n_groups):
                egs = list(range(egg * EGROUP, min((egg + 1) * EGROUP, N_ETILES)))
                W = len(egs) * ETILE
                base = egs[0] * ETILE
                et_t = etp.tile([P, 2, W], bf16, tag="et")
                nc.sync.dma_start(et_t[:], et_v[:, :, base:base + W])
                ebc_t = ebcp.tile([P, W], f32, tag="ebc")
                nc.gpsimd.partition_broadcast(ebc_t[:], entsq_sb[0:1, base:base + W])
                ots = [outp.tile([P, W], f32, tag="ot", name=f"ot{egg}_{bt}")
                       for bt in range(N_BTILES)]
                for j, eg in enumerate(egs):
                    jsl = slice(j * ETILE, (j + 1) * ETILE)
                    for bt in range(N_BTILES):
                        bsl = slice(bt * P, (bt + 1) * P)
                        ps = psp.tile([P, ETILE], f32, tag="ps")
                        nc.tensor.matmul(ps[:], hrt_sb[:, 0, bsl],
                                         et_t[:, 0, jsl],
                                         start=True, stop=False)
                        nc.tensor.matmul(ps[:], hrt_sb[:, 1, bsl],
                                         et_t[:, 1, jsl],
                                         start=False, stop=True)
                        nc.vector.scalar_tensor_tensor(
                            ots[bt][:, jsl], ps[:], hrsq_sb[:, bt:bt + 1],
                            ebc_t[:, jsl],
                            op0=mybir.AluOpType.subtract,
                            op1=mybir.AluOpType.subtract,
                        )
                for bt in range(N_BTILES):
                    bsl = slice(bt * P, (bt + 1) * P)
                    nc.sync.dma_start(scores[bsl, base:base + W], ots[bt][:])
    nc.compile()
    return nc


def _get_bass():
    if "nc" not in _compiled:
        _compiled["nc"] = _build_bass()
    return _compiled["nc"]


def _device_scores(E, hr, hr_sq, ent_sq, trace=False):
    """Run the sharded GEMM on 8 cores; returns (scores [B, N_ENT], results)."""
    bf = ml_dtypes.bfloat16
    hrt2 = np.ascontiguousarray((2.0 * hr).T).astype(bf)         # [256, 1024]
    hrsq_t = np.ascontiguousarray(hr_sq.reshape(N_BTILES, P).T)  # [128, 8]
    in_maps = []
    for c in range(N_CORES):
        Ec = E[c * N_SH:(c + 1) * N_SH]
        in_maps.append({
            "et": np.ascontiguousarray(Ec.T).astype(bf),          # [256, 12500]
            "hrt2": hrt2,
            "hrsq": hrsq_t,
            "entsq": np.ascontiguousarray(
                ent_sq[c * N_SH:(c + 1) * N_SH].reshape(1, N_SH)),
        })
    nc = _get_bass()
    res = run_bass_kernel_spmd(nc, in_maps, core_ids=list(range(N_CORES)),
                               trace=trace)
    scores = np.concatenate([r["scores"] for r in res.results], axis=1)
    return scores, res


def _refine_predictions(scores, hr, E, hr_sq, ent_sq):
    """argmax with exact rescoring of near-max candidates.

    Device scores carry bf16/fp16 input-rounding noise (~0.3 abs). Any
    entity within MARGIN of a query's device max is rescored exactly in
    fp64 so the returned argmax matches an exact computation.
    """
    MARGIN = 2.0
    amax = scores.max(axis=1)
    preds = np.empty(B, dtype=np.int64)
    hr64 = hr.astype(np.float64)
    for b in range(B):
        cand = np.flatnonzero(scores[b] >= amax[b] - MARGIN)
        if cand.size == 1:
            preds[b] = cand[0]
            continue
        s = 2.0 * (E[cand].astype(np.float64) @ hr64[b]) \
            - hr_sq[b] - ent_sq[cand]
        preds[b] = cand[int(np.argmax(s))]
    return preds


def kernel(queries, entity_emb, relation_emb, trace=False):
    queries = np.asarray(queries)
    E = np.ascontiguousarray(np.asarray(entity_emb, dtype=np.float32))
    R = np.ascontiguousarray(np.asarray(relation_emb, dtype=np.float32))
    heads = queries[:, 0].astype(np.int64)
    rels = queries[:, 1].astype(np.int64)

    hr = E[heads] + R[rels]                         # [B, D] f32
    hr_sq = np.sum(hr * hr, axis=1)                 # [B] f32
    ent_sq = np.sum(E * E, axis=1)                  # [N] f32

    scores, res = _device_scores(E, hr, hr_sq, ent_sq, trace=trace)
    preds = _refine_predictions(scores, hr, E, hr_sq, ent_sq)
    if trace:
        kernel.last_results = res
    return scores, preds
